# revision 63
# baseline (speedup 1.0000x reference)
"""Local windowed multi-head attention (lucidrains-style, causal, look_backward=1)
on 8 Trainium2 NeuronCores.

Sharding: core = (batch b in {0,1}) x (1024-token chunk c in {0..3}).
Each core computes its chunk's full output rows independently (local attention
only needs a 512-token K/V halo from the previous chunk), so the host-side
unshard is a pure concatenation - no collectives.

Per-core pipeline (all matmuls bf16 with f32 PSUM accumulation), per group of
4 heads:
  proj:  kT[h] [d,1536] and qT[h] [d,1024] via weight-stationary matmuls
         (scale folded into Wq host-side); V projected directly in natural
         layout v_nat[t][tok,4*128d] via x-stationary matmuls (no transposes).
  attn (per head, window): dots computed TRANSPOSED dT[kv,q] = kT_tile.T @ qT
         so softmax needs no PE transposes; exp on Act engine straight out of
         PSUM (no max subtraction - logits are O(5); first-window halo killed
         via a per-core -inf activation bias); causal diagonal masked with one
         [128,128] bf16 multiply; row sums via a bf16 tree-add on DVE plus a
         gpsimd cross-partition add; aoT = v_nat.T @ expT accumulated in PSUM,
         normalized by one DVE multiply with the broadcast reciprocal.
  out:   out^T[f,tok] = WoT_tile.T @ aoT + bo from SBUF-resident aoT.
"""
import sys
sys.path.insert(0, "/opt/trn_rl_repo")

import numpy as np
import ml_dtypes

import concourse.bass as bass
import concourse.bass_isa as bass_isa
import concourse.tile as tile
import concourse.mybir as mybir
from concourse import bacc, bass_utils

S, B, E, H, D = 4096, 2, 2048, 16, 128
WIN = 512
CHUNK = 1024          # q tokens per core
HALO = 512            # k/v lookback
TOK = HALO + CHUNK    # 1536 kv tokens per core
ET = E // 128         # 16 e-tiles
NW = CHUNK // WIN     # 2 windows per core
HG = 4                # heads per group
NG = H // HG          # 4 groups
NEG = -3.0e38
SCALE = D ** -0.5
F32 = mybir.dt.float32
BF16 = mybir.dt.bfloat16
BF = ml_dtypes.bfloat16


def _build():
    nc = bacc.Bacc("TRN2", target_bir_lowering=False, debug=False)
    dt = nc.dram_tensor
    xq_d = dt("xq", [E, CHUNK], BF16, kind="ExternalInput").ap()
    xk_d = dt("xk", [E, TOK], BF16, kind="ExternalInput").ap()
    xv_d = dt("xv", [E, TOK], BF16, kind="ExternalInput").ap()
    wq_d = dt("wq", [128, H, E], BF16, kind="ExternalInput").ap()   # [p,h,et*128+d]
    wk_d = dt("wk", [128, H, E], BF16, kind="ExternalInput").ap()
    wv_d = dt("wv", [E, E], BF16, kind="ExternalInput").ap()        # Wv.T
    wo_d = dt("wo", [128, ET, E], BF16, kind="ExternalInput").ap()  # [p,ft,et*128+f]
    bo_d = dt("bo", [128, ET], F32, kind="ExternalInput").ap()
    tri_d = dt("tri", [128, 128], BF16, kind="ExternalInput").ap()
    w0b_d = dt("w0b", [128, 1], F32, kind="ExternalInput").ap()
    out_d = dt("out", [E, CHUNK], F32, kind="ExternalOutput").ap()

    with tile.TileContext(nc) as tc:
        with tc.tile_pool(name="const", bufs=1) as cpool, \
             tc.tile_pool(name="xp", bufs=3) as xpool, \
             tc.tile_pool(name="wp", bufs=4) as wpool, \
             tc.tile_pool(name="wvp", bufs=1) as wvpool, \
             tc.tile_pool(name="ktp", bufs=2 * HG) as ktpool, \
             tc.tile_pool(name="qtp", bufs=6) as qtpool, \
             tc.tile_pool(name="vnp", bufs=24) as vnpool, \
             tc.tile_pool(name="aop", bufs=H) as aopool, \
             tc.tile_pool(name="expp", bufs=12) as expool, \
             tc.tile_pool(name="sacc", bufs=3) as saccpool, \
             tc.tile_pool(name="rbp", bufs=2) as rbpool, \
             tc.tile_pool(name="outp", bufs=2) as outpool, \
             tc.tile_pool(name="psP", bufs=3, space="PSUM") as psP, \
             tc.tile_pool(name="psD", bufs=3, space="PSUM") as psD, \
             tc.tile_pool(name="psV", bufs=2, space="PSUM") as psV:

            def load_xblk(src, blk, tag):
                """Load x^T block [128, 16et, 512tok] as 4 sub-DMAs so the
                first proj matmuls can start after ~1/4 of the transfer."""
                xb = xpool.tile([128, ET, 512], BF16, tag="x", name=tag)
                for q in range(4):
                    nc.sync.dma_start(
                        xb[:, q * 4:(q + 1) * 4, :],
                        src[q * 512:(q + 1) * 512, blk * 512:(blk + 1) * 512]
                        .rearrange("(t p) c -> p t c", p=128))
                return xb

            consts_loaded = False
            wogs = []
            pss_split = []

            def load_consts():
                t = cpool.tile([128, 128], BF16, tag="tri")
                nc.sync.dma_start(t[:], tri_d)
                w = cpool.tile([128, 1], F32, tag="w0b")
                nc.sync.dma_start(w[:], w0b_d)
                b = cpool.tile([128, ET], F32, tag="bo")
                nc.sync.dma_start(b[:], bo_d)
                return t, w, b

            aots = []
            for g in range(NG):
                heads = list(range(g * HG, (g + 1) * HG))
                # ---- K projection: kT[h] = [d=128, TOK] ----
                kts = {}
                wk_h = {}

                def load_wk(h):
                    w = wpool.tile([128, E], BF16, tag="w", name=f"wk{h}")
                    if h == 0:
                        # first weight: split the DMA so the first Ldweights
                        # only waits on a quarter of the transfer
                        for q in range(4):
                            nc.sync.dma_start(w[:, q * 512:(q + 1) * 512],
                                              wk_d[:, h, q * 512:(q + 1) * 512])
                    else:
                        nc.sync.dma_start(w[:], wk_d[:, h, :])
                    wk_h[h] = w
                    kts[h] = ktpool.tile([128, TOK], BF16, tag="kt", name=f"kt{h}")

                xb0 = None
                if g == 0:
                    # startup: the very first matmul needs wk0[:, :128] and
                    # xk et-tile 0 only -- put exactly those two transfers at
                    # the head of the DMA queue
                    w0 = wpool.tile([128, E], BF16, tag="w", name="wk0")
                    nc.sync.dma_start(w0[:, 0:512], wk_d[:, 0, 0:512])
                    xb0 = xpool.tile([128, ET, 512], BF16, tag="x", name="xk0_0")
                    nc.sync.dma_start(
                        xb0[:, 0:4, :],
                        xk_d[0:512, 0:512].rearrange("(t p) c -> p t c", p=128))
                    for q in range(1, 4):
                        nc.sync.dma_start(w0[:, q * 512:(q + 1) * 512],
                                          wk_d[:, 0, q * 512:(q + 1) * 512])
                        nc.sync.dma_start(
                            xb0[:, q * 4:(q + 1) * 4, :],
                            xk_d[q * 512:(q + 1) * 512, 0:512]
                            .rearrange("(t p) c -> p t c", p=128))
                    wk_h[0] = w0
                    kts[0] = ktpool.tile([128, TOK], BF16, tag="kt", name="kt0")
                    for h in heads[1:]:
                        load_wk(h)
                else:
                    for h in heads:
                        load_wk(h)
                for blk in range(TOK // 512):
                    xb = xb0 if (g == 0 and blk == 0) else \
                        load_xblk(xk_d, blk, f"xk{g}_{blk}")
                    if not consts_loaded:
                        tri, w0b, bo_sb = load_consts()
                        consts_loaded = True
                    if g == 0 and blk == 0:
                        # startup: the first chain races the x-transfer
                        # stream; interleave three head-chains quarter-major
                        # so each arriving x sub-block feeds 12 matmuls
                        # instead of 4
                        pss3 = [psP.tile([128, 512], F32, tag="proj",
                                         name=f"pk3_{i}") for i in range(3)]
                        for q in range(4):
                            for hi in range(3):
                                for et in range(q * 4, q * 4 + 4):
                                    nc.tensor.matmul(
                                        pss3[hi][:],
                                        wk_h[heads[hi]][:, et * 128:(et + 1) * 128],
                                        xb[:, et, :],
                                        start=(et == 0), stop=(et == ET - 1),
                                        skip_group_check=True)
                        for hi in range(3):
                            nc.vector.tensor_copy(kts[heads[hi]][:, 0:512],
                                                  pss3[hi][:])
                        rest = heads[3:]
                    else:
                        rest = heads
                    for h in rest:
                        ps = psP.tile([128, 512], F32, tag="proj")
                        for et in range(ET):
                            nc.tensor.matmul(ps[:], wk_h[h][:, et * 128:(et + 1) * 128],
                                             xb[:, et, :],
                                             start=(et == 0), stop=(et == ET - 1))
                        nc.vector.tensor_copy(kts[h][:, blk * 512:(blk + 1) * 512], ps[:])
                # ---- V projection (natural layout): v_nat[t] = [tok=128, HG*128] ----
                wvg = wvpool.tile([128, ET, 512], BF16, tag="wv", name=f"wv{g}")
                nc.sync.dma_start(
                    wvg[:],
                    wv_d[:, g * 512:(g + 1) * 512]
                    .rearrange("(t p) f -> p t f", p=128))
                vns = [vnpool.tile([128, HG * 128], BF16, tag="vn", name=f"vn{g}_{t}")
                       for t in range(TOK // 128)]
                for blk in range(TOK // 512):
                    xb = load_xblk(xv_d, blk, f"xv{g}_{blk}")
                    for t4 in range(4):
                        t = blk * 4 + t4
                        ps = psP.tile([128, 512], F32, tag="proj")
                        for et in range(ET):
                            nc.tensor.matmul(ps[:], xb[:, et, t4 * 128:(t4 + 1) * 128],
                                             wvg[:, et, :],
                                             start=(et == 0), stop=(et == ET - 1))
                        nc.vector.tensor_copy(vns[t][:], ps[:])
                # ---- Q projection: qT[h] = [d=128, CHUNK] (scale folded in Wq) ----
                qts = {}
                wq_h = {}
                for h in heads:
                    w = wpool.tile([128, E], BF16, tag="w", name=f"wq{h}")
                    nc.sync.dma_start(w[:], wq_d[:, h, :])
                    wq_h[h] = w
                    qts[h] = qtpool.tile([128, CHUNK], BF16, tag="qt", name=f"qt{h}")
                for blk in range(CHUNK // 512):
                    xb = load_xblk(xq_d, blk, f"xq{g}_{blk}")
                    for h in heads:
                        ps = psP.tile([128, 512], F32, tag="proj")
                        for et in range(ET):
                            nc.tensor.matmul(ps[:], wq_h[h][:, et * 128:(et + 1) * 128],
                                             xb[:, et, :],
                                             start=(et == 0), stop=(et == ET - 1))
                        nc.vector.tensor_copy(qts[h][:, blk * 512:(blk + 1) * 512], ps[:])
                if g == NG - 1:
                    # prefetch Wo during the last group's attention; ft=0
                    # gets a dedicated tile so the split chains' late reads
                    # don't stall the rotating weight pool
                    for ft in range(ET):
                        if ft == 0:
                            wog = cpool.tile([128, E], BF16, tag="wo0",
                                             name="wo0")
                        else:
                            wog = wpool.tile([128, E], BF16, tag="w",
                                             name=f"wo{ft}")
                        nc.sync.dma_start(wog[:], wo_d[:, ft, :])
                        wogs.append(wog)
                    # out-proj heads 0..11 for the first two chains only needs
                    # groups 0-2's outputs; dribble those matmuls into this
                    # group's attention stream to fill PE bubbles
                    split = ET - 4
                    for blk in range(2):
                        pss_split.append(psP.tile([128, 512], F32, tag="proj",
                                                  name=f"osplit{blk}"))

                    def _fill_gen():
                        for blk in range(2):
                            for et in range(split):
                                yield (pss_split[blk], blk, et)

                    fill_iter = _fill_gen()

                    def fill(n):
                        for _ in range(n):
                            item = next(fill_iter, None)
                            if item is None:
                                return
                            ps, blk, et = item
                            nc.tensor.matmul(
                                ps[:], wogs[0][:, et * 128:(et + 1) * 128],
                                aots[et][:, blk * 512:(blk + 1) * 512],
                                start=(et == 0), stop=False,
                                skip_group_check=True)
                else:
                    def fill(n):
                        pass
                # ---- attention ----
                for h in heads:
                    hi = h - g * HG
                    aot = aopool.tile([128, CHUNK], BF16, tag="aot", name=f"aot{h}")
                    aots.append(aot)
                    for w in range(NW):
                        exps = []
                        for jb in range(8):
                            qlo = max(0, (jb - 4) * 128)
                            ncol = 512 - qlo
                            pd = psD.tile([128, 512], F32, tag="dt")
                            nc.tensor.matmul(
                                pd[:, :ncol],
                                kts[h][:, w * 512 + jb * 128: w * 512 + (jb + 1) * 128],
                                qts[h][:, w * 512 + qlo: w * 512 + 512],
                                start=True, stop=True)
                            ex = expool.tile([128, 512], BF16, tag="exp")
                            bias = w0b[:] if (w == 0 and jb < 4) else 0.0
                            nc.scalar.activation(ex[:, :ncol], pd[:, :ncol],
                                                 mybir.ActivationFunctionType.Exp,
                                                 bias=bias, scale=1.0)
                            if jb >= 4:
                                nc.vector.tensor_mul(ex[:, 0:128], ex[:, 0:128], tri[:])
                            exps.append((ex, qlo, ncol))
                        if g == NG - 1 and not (h == heads[0] and w == 0):
                            fill(4)
                        # row sums over kv: bf16 tree-add on DVE, then
                        # cross-partition add on gpsimd (frees the PE).
                        s01 = saccpool.tile([128, 512], BF16, tag="sacc")
                        nc.vector.tensor_add(s01[:], exps[0][0][:], exps[1][0][:])
                        s23 = saccpool.tile([128, 512], BF16, tag="sacc")
                        nc.vector.tensor_add(s23[:], exps[2][0][:], exps[3][0][:])
                        acc = saccpool.tile([128, 512], BF16, tag="sacc")
                        nc.vector.tensor_add(acc[:], s01[:], s23[:])
                        nc.vector.tensor_add(acc[:], acc[:], exps[4][0][:])
                        for jb in (5, 6, 7):
                            qlo = (jb - 4) * 128
                            nc.vector.tensor_add(acc[:, qlo:512], acc[:, qlo:512],
                                                 exps[jb][0][:, :512 - qlo])
                        sm = rbpool.tile([128, 512], F32, tag="sm")
                        nc.gpsimd.partition_all_reduce(sm[:], acc[:], 128,
                                                       bass_isa.ReduceOp.add)
                        rb = rbpool.tile([128, 512], F32, tag="rb")
                        nc.vector.reciprocal(rb[:], sm[:])
                        pav = psV.tile([128, 512], F32, tag="av")
                        for jb, (ex, qlo, ncol) in enumerate(exps):
                            t = w * 4 + jb
                            nc.tensor.matmul(pav[:, qlo:512],
                                             vns[t][:, hi * 128:(hi + 1) * 128],
                                             ex[:, :ncol],
                                             start=(jb == 0), stop=(jb == 7),
                                             skip_group_check=True)
                        nc.vector.tensor_mul(aot[:, w * 512:(w + 1) * 512], pav[:], rb[:])
            # ---- output projection (weights prefetched during last group) ----
            def out_tail(ft, blk, ps):
                ob = outpool.tile([128, 512], F32, tag="ob")
                nc.scalar.activation(ob[:], ps[:],
                                     mybir.ActivationFunctionType.Identity,
                                     bias=bo_sb[:, ft:ft + 1], scale=1.0)
                nc.sync.dma_start(
                    out_d[ft * 128:(ft + 1) * 128, blk * 512:(blk + 1) * 512], ob[:])

            # finish the split chains; the w0-half tail is ready ~4.5us
            # before the w1 half, so slot two w0-only chains in between to
            # keep the in-order PE stream busy across that window
            split = ET - 4
            for et in range(split, ET):
                nc.tensor.matmul(pss_split[0][:], wogs[0][:, et * 128:(et + 1) * 128],
                                 aots[et][:, 0:512],
                                 start=False, stop=(et == ET - 1),
                                 skip_group_check=True)
            out_tail(0, 0, pss_split[0])
            for ft in (1, 2, 3):
                ps = psP.tile([128, 512], F32, tag="proj", name=f"po{ft}b0")
                for et in range(ET):
                    nc.tensor.matmul(ps[:], wogs[ft][:, et * 128:(et + 1) * 128],
                                     aots[et][:, 0:512],
                                     start=(et == 0), stop=(et == ET - 1))
                out_tail(ft, 0, ps)
            for et in range(split, ET):
                nc.tensor.matmul(pss_split[1][:], wogs[0][:, et * 128:(et + 1) * 128],
                                 aots[et][:, 512:1024],
                                 start=False, stop=(et == ET - 1),
                                 skip_group_check=True)
            out_tail(0, 1, pss_split[1])
            for ft in range(1, ET):
                wog = wogs[ft]
                for blk in range(CHUNK // 512):
                    if blk == 0 and ft in (1, 2, 3):
                        continue
                    if ft == ET - 1 and blk == CHUNK // 512 - 1:
                        # final tile: two half-column chains so the first
                        # half's bias-add + store overlap the second half's
                        # matmuls instead of serializing after them
                        for col in range(2):
                            ps = psD.tile([128, 256], F32, tag="dt",
                                          name=f"pslast{col}")
                            cl = blk * 512 + col * 256
                            for et in range(ET):
                                nc.tensor.matmul(
                                    ps[:], wog[:, et * 128:(et + 1) * 128],
                                    aots[et][:, cl:cl + 256],
                                    start=(et == 0), stop=(et == ET - 1))
                            ob = outpool.tile([128, 256], F32, tag="ob",
                                              name=f"oblast{col}")
                            nc.scalar.activation(
                                ob[:], ps[:],
                                mybir.ActivationFunctionType.Identity,
                                bias=bo_sb[:, ft:ft + 1], scale=1.0)
                            nc.sync.dma_start(
                                out_d[ft * 128:(ft + 1) * 128, cl:cl + 256], ob[:])
                        continue
                    ps = psP.tile([128, 512], F32, tag="proj")
                    for et in range(ET):
                        nc.tensor.matmul(ps[:], wog[:, et * 128:(et + 1) * 128],
                                         aots[et][:, blk * 512:(blk + 1) * 512],
                                         start=(et == 0), stop=(et == ET - 1))
                    out_tail(ft, blk, ps)
    nc.compile()
    return nc


_NC_CACHE = None
_LAST_IN_MAPS = None


def _pack_whead(wt, scale=1.0):
    # wt = W.T as [E, E] f32; -> [128, H, E] with [p, h, et*128+d] = wt[et*128+p, h*128+d]
    r = (wt * scale).reshape(ET, 128, H, D).transpose(1, 2, 0, 3).reshape(128, H, E)
    return np.ascontiguousarray(r).astype(BF)


def kernel(query, key, value, input_mask, Wq, Wk, Wv, Wo, bo):
    global _NC_CACHE, _LAST_IN_MAPS
    if _NC_CACHE is None:
        _NC_CACHE = _build()
    nc = _NC_CACHE

    wq = _pack_whead(np.asarray(Wq, np.float32).T, SCALE)
    wk = _pack_whead(np.asarray(Wk, np.float32).T)
    wv = np.ascontiguousarray(np.asarray(Wv, np.float32).T).astype(BF)
    wo = _pack_whead(np.asarray(Wo, np.float32).T)          # [128, ft, et*128+f]
    bo_t = np.ascontiguousarray(
        np.asarray(bo, np.float32).reshape(ET, 128).T)      # [128, ET]
    tri = np.triu(np.ones((128, 128), np.float32)).astype(BF)

    in_maps = []
    for core in range(8):
        b, c = core // 4, core % 4
        lo, hi = c * CHUNK, (c + 1) * CHUNK
        xq = np.asarray(query[lo:hi, b, :], np.float32)     # [1024, E]
        xkv_k = np.zeros((TOK, E), np.float32)
        xkv_v = np.zeros((TOK, E), np.float32)
        klo = max(lo - HALO, 0)
        xkv_k[HALO - (lo - klo):] = np.asarray(key[klo:hi, b, :], np.float32)
        xkv_v[HALO - (lo - klo):] = np.asarray(value[klo:hi, b, :], np.float32)
        w0b = np.full((128, 1), NEG if c == 0 else 0.0, np.float32)
        in_maps.append({
            "xq": np.ascontiguousarray(xq.T).astype(BF),
            "xk": np.ascontiguousarray(xkv_k.T).astype(BF),
            "xv": np.ascontiguousarray(xkv_v.T).astype(BF),
            "wq": wq, "wk": wk, "wv": wv, "wo": wo,
            "bo": bo_t, "tri": tri, "w0b": w0b,
        })

    _LAST_IN_MAPS = in_maps
    res = bass_utils.run_bass_kernel_spmd(nc, in_maps, core_ids=list(range(8)))
    out = np.empty((S, B, E), np.float32)
    for core in range(8):
        b, c = core // 4, core % 4
        out[c * CHUNK:(c + 1) * CHUNK, b, :] = res.results[core]["out"].T
    return out


# revision 64
# speedup vs baseline: 1.0107x; 1.0107x over previous
"""Local windowed multi-head attention (lucidrains-style, causal, look_backward=1)
on 8 Trainium2 NeuronCores.

Sharding: core = (batch b in {0,1}) x (1024-token chunk c in {0..3}).
Each core computes its chunk's full output rows independently (local attention
only needs a 512-token K/V halo from the previous chunk), so the host-side
unshard is a pure concatenation - no collectives.

Per-core pipeline (all matmuls bf16 with f32 PSUM accumulation), per group of
4 heads:
  proj:  kT[h] [d,1536] and qT[h] [d,1024] via weight-stationary matmuls
         (scale folded into Wq host-side); V projected directly in natural
         layout v_nat[t][tok,4*128d] via x-stationary matmuls (no transposes).
  attn (per head, window): dots computed TRANSPOSED dT[kv,q] = kT_tile.T @ qT
         so softmax needs no PE transposes; exp on Act engine straight out of
         PSUM (no max subtraction - logits are O(5); first-window halo killed
         via a per-core -inf activation bias); causal diagonal masked with one
         [128,128] bf16 multiply; row sums via a bf16 tree-add on DVE plus a
         gpsimd cross-partition add; aoT = v_nat.T @ expT accumulated in PSUM,
         normalized by one DVE multiply with the broadcast reciprocal.
  out:   out^T[f,tok] = WoT_tile.T @ aoT + bo from SBUF-resident aoT.
"""
import sys
sys.path.insert(0, "/opt/trn_rl_repo")

import numpy as np
import ml_dtypes

import concourse.bass as bass
import concourse.bass_isa as bass_isa
import concourse.tile as tile
import concourse.mybir as mybir
from concourse import bacc, bass_utils

S, B, E, H, D = 4096, 2, 2048, 16, 128
WIN = 512
CHUNK = 1024          # q tokens per core
HALO = 512            # k/v lookback
TOK = HALO + CHUNK    # 1536 kv tokens per core
ET = E // 128         # 16 e-tiles
NW = CHUNK // WIN     # 2 windows per core
HG = 4                # heads per group
NG = H // HG          # 4 groups
NEG = -3.0e38
SCALE = D ** -0.5
F32 = mybir.dt.float32
BF16 = mybir.dt.bfloat16
BF = ml_dtypes.bfloat16


def _build():
    nc = bacc.Bacc("TRN2", target_bir_lowering=False, debug=False)
    dt = nc.dram_tensor
    xq_d = dt("xq", [E, CHUNK], BF16, kind="ExternalInput").ap()
    xk_d = dt("xk", [E, TOK], BF16, kind="ExternalInput").ap()
    xv_d = dt("xv", [E, TOK], BF16, kind="ExternalInput").ap()
    wq_d = dt("wq", [128, H, E], BF16, kind="ExternalInput").ap()   # [p,h,et*128+d]
    wk_d = dt("wk", [128, H, E], BF16, kind="ExternalInput").ap()
    wv_d = dt("wv", [E, E], BF16, kind="ExternalInput").ap()        # Wv.T
    wo_d = dt("wo", [128, ET, E], BF16, kind="ExternalInput").ap()  # [p,ft,et*128+f]
    bo_d = dt("bo", [128, ET], F32, kind="ExternalInput").ap()
    tri_d = dt("tri", [128, 128], BF16, kind="ExternalInput").ap()
    w0b_d = dt("w0b", [128, 1], F32, kind="ExternalInput").ap()
    out_d = dt("out", [E, CHUNK], F32, kind="ExternalOutput").ap()

    with tile.TileContext(nc) as tc:
        with tc.tile_pool(name="const", bufs=1) as cpool, \
             tc.tile_pool(name="xp", bufs=3) as xpool, \
             tc.tile_pool(name="wp", bufs=4) as wpool, \
             tc.tile_pool(name="wvp", bufs=1) as wvpool, \
             tc.tile_pool(name="ktp", bufs=2 * HG) as ktpool, \
             tc.tile_pool(name="qtp", bufs=6) as qtpool, \
             tc.tile_pool(name="vnp", bufs=24) as vnpool, \
             tc.tile_pool(name="aop", bufs=H) as aopool, \
             tc.tile_pool(name="expp", bufs=12) as expool, \
             tc.tile_pool(name="sacc", bufs=3) as saccpool, \
             tc.tile_pool(name="rbp", bufs=2) as rbpool, \
             tc.tile_pool(name="outp", bufs=2) as outpool, \
             tc.tile_pool(name="psP", bufs=3, space="PSUM") as psP, \
             tc.tile_pool(name="psD", bufs=3, space="PSUM") as psD, \
             tc.tile_pool(name="psV", bufs=2, space="PSUM") as psV:

            def load_xblk(src, blk, tag):
                """Load x^T block [128, 16et, 512tok] as 4 sub-DMAs so the
                first proj matmuls can start after ~1/4 of the transfer."""
                xb = xpool.tile([128, ET, 512], BF16, tag="x", name=tag)
                for q in range(4):
                    nc.sync.dma_start(
                        xb[:, q * 4:(q + 1) * 4, :],
                        src[q * 512:(q + 1) * 512, blk * 512:(blk + 1) * 512]
                        .rearrange("(t p) c -> p t c", p=128))
                return xb

            consts_loaded = False
            wogs = []
            pss_split = []

            def load_consts():
                t = cpool.tile([128, 128], BF16, tag="tri")
                nc.sync.dma_start(t[:], tri_d)
                w = cpool.tile([128, 1], F32, tag="w0b")
                nc.sync.dma_start(w[:], w0b_d)
                b = cpool.tile([128, ET], F32, tag="bo")
                nc.sync.dma_start(b[:], bo_d)
                return t, w, b

            aots = []
            for g in range(NG):
                heads = list(range(g * HG, (g + 1) * HG))
                # ---- K projection: kT[h] = [d=128, TOK] ----
                kts = {}
                wk_h = {}

                def load_wk(h):
                    w = wpool.tile([128, E], BF16, tag="w", name=f"wk{h}")
                    if h == 0:
                        # first weight: split the DMA so the first Ldweights
                        # only waits on a quarter of the transfer
                        for q in range(4):
                            nc.sync.dma_start(w[:, q * 512:(q + 1) * 512],
                                              wk_d[:, h, q * 512:(q + 1) * 512])
                    else:
                        nc.sync.dma_start(w[:], wk_d[:, h, :])
                    wk_h[h] = w
                    kts[h] = ktpool.tile([128, TOK], BF16, tag="kt", name=f"kt{h}")

                xb0 = None
                if g == 0:
                    # startup: the very first matmul needs wk0[:, :128] and
                    # xk et-tile 0 only -- put exactly those two transfers at
                    # the head of the DMA queue
                    w0 = wpool.tile([128, E], BF16, tag="w", name="wk0")
                    nc.sync.dma_start(w0[:, 0:512], wk_d[:, 0, 0:512])
                    xb0 = xpool.tile([128, ET, 512], BF16, tag="x", name="xk0_0")
                    nc.sync.dma_start(
                        xb0[:, 0:4, :],
                        xk_d[0:512, 0:512].rearrange("(t p) c -> p t c", p=128))
                    for q in range(1, 4):
                        nc.sync.dma_start(w0[:, q * 512:(q + 1) * 512],
                                          wk_d[:, 0, q * 512:(q + 1) * 512])
                        nc.sync.dma_start(
                            xb0[:, q * 4:(q + 1) * 4, :],
                            xk_d[q * 512:(q + 1) * 512, 0:512]
                            .rearrange("(t p) c -> p t c", p=128))
                    wk_h[0] = w0
                    kts[0] = ktpool.tile([128, TOK], BF16, tag="kt", name="kt0")
                    for h in heads[1:]:
                        load_wk(h)
                else:
                    for h in heads:
                        load_wk(h)
                for blk in range(TOK // 512):
                    xb = xb0 if (g == 0 and blk == 0) else \
                        load_xblk(xk_d, blk, f"xk{g}_{blk}")
                    if not consts_loaded:
                        tri, w0b, bo_sb = load_consts()
                        consts_loaded = True
                    if g == 0 and blk == 0:
                        # startup: the first chain races the x-transfer
                        # stream; interleave three head-chains quarter-major
                        # so each arriving x sub-block feeds 12 matmuls
                        # instead of 4
                        pss3 = [psP.tile([128, 512], F32, tag="proj",
                                         name=f"pk3_{i}") for i in range(3)]
                        for q in range(4):
                            for hi in range(3):
                                for et in range(q * 4, q * 4 + 4):
                                    nc.tensor.matmul(
                                        pss3[hi][:],
                                        wk_h[heads[hi]][:, et * 128:(et + 1) * 128],
                                        xb[:, et, :],
                                        start=(et == 0), stop=(et == ET - 1),
                                        skip_group_check=True)
                        for hi in range(3):
                            nc.vector.tensor_copy(kts[heads[hi]][:, 0:512],
                                                  pss3[hi][:])
                        rest = heads[3:]
                    else:
                        rest = heads
                    for h in rest:
                        ps = psP.tile([128, 512], F32, tag="proj")
                        for et in range(ET):
                            nc.tensor.matmul(ps[:], wk_h[h][:, et * 128:(et + 1) * 128],
                                             xb[:, et, :],
                                             start=(et == 0), stop=(et == ET - 1))
                        nc.vector.tensor_copy(kts[h][:, blk * 512:(blk + 1) * 512], ps[:])
                # ---- V projection (natural layout): v_nat[t] = [tok=128, HG*128] ----
                wvg = wvpool.tile([128, ET, 512], BF16, tag="wv", name=f"wv{g}")
                nc.sync.dma_start(
                    wvg[:],
                    wv_d[:, g * 512:(g + 1) * 512]
                    .rearrange("(t p) f -> p t f", p=128))
                vns = [vnpool.tile([128, HG * 128], BF16, tag="vn", name=f"vn{g}_{t}")
                       for t in range(TOK // 128)]
                for blk in range(2):
                    xb = load_xblk(xv_d, blk, f"xv{g}_{blk}")
                    for t4 in range(4):
                        t = blk * 4 + t4
                        ps = psP.tile([128, 512], F32, tag="proj")
                        for et in range(ET):
                            nc.tensor.matmul(ps[:], xb[:, et, t4 * 128:(t4 + 1) * 128],
                                             wvg[:, et, :],
                                             start=(et == 0), stop=(et == ET - 1))
                        nc.vector.tensor_copy(vns[t][:], ps[:])
                # t8..11 are only read by window-1 attention; defer those four
                # chains into the window-0 sweep so they overlap the Act
                # engine's exp burst instead of serializing before it
                xb2 = load_xblk(xv_d, 2, f"xv{g}_2")

                def defer_vchain(t4):
                    ps = psP.tile([128, 512], F32, tag="proj",
                                  name=f"vd{g}_{t4}")
                    for et in range(ET):
                        nc.tensor.matmul(ps[:], xb2[:, et, t4 * 128:(t4 + 1) * 128],
                                         wvg[:, et, :],
                                         start=(et == 0), stop=(et == ET - 1))
                    nc.vector.tensor_copy(vns[8 + t4][:], ps[:])
                # ---- Q projection: qT[h] = [d=128, CHUNK] (scale folded in Wq) ----
                qts = {}
                wq_h = {}
                for h in heads:
                    w = wpool.tile([128, E], BF16, tag="w", name=f"wq{h}")
                    nc.sync.dma_start(w[:], wq_d[:, h, :])
                    wq_h[h] = w
                    qts[h] = qtpool.tile([128, CHUNK], BF16, tag="qt", name=f"qt{h}")
                for blk in range(CHUNK // 512):
                    xb = load_xblk(xq_d, blk, f"xq{g}_{blk}")
                    for h in heads:
                        ps = psP.tile([128, 512], F32, tag="proj")
                        for et in range(ET):
                            nc.tensor.matmul(ps[:], wq_h[h][:, et * 128:(et + 1) * 128],
                                             xb[:, et, :],
                                             start=(et == 0), stop=(et == ET - 1))
                        nc.vector.tensor_copy(qts[h][:, blk * 512:(blk + 1) * 512], ps[:])
                if g == NG - 1:
                    # prefetch Wo during the last group's attention; ft=0
                    # gets a dedicated tile so the split chains' late reads
                    # don't stall the rotating weight pool
                    for ft in range(ET):
                        if ft == 0:
                            wog = cpool.tile([128, E], BF16, tag="wo0",
                                             name="wo0")
                        else:
                            wog = wpool.tile([128, E], BF16, tag="w",
                                             name=f"wo{ft}")
                        nc.sync.dma_start(wog[:], wo_d[:, ft, :])
                        wogs.append(wog)
                    # out-proj heads 0..11 for the first two chains only needs
                    # groups 0-2's outputs; dribble those matmuls into this
                    # group's attention stream to fill PE bubbles
                    split = ET - 4
                    for blk in range(2):
                        pss_split.append(psP.tile([128, 512], F32, tag="proj",
                                                  name=f"osplit{blk}"))

                    def _fill_gen():
                        for blk in range(2):
                            for et in range(split):
                                yield (pss_split[blk], blk, et)

                    fill_iter = _fill_gen()

                    def fill(n):
                        for _ in range(n):
                            item = next(fill_iter, None)
                            if item is None:
                                return
                            ps, blk, et = item
                            nc.tensor.matmul(
                                ps[:], wogs[0][:, et * 128:(et + 1) * 128],
                                aots[et][:, blk * 512:(blk + 1) * 512],
                                start=(et == 0), stop=False,
                                skip_group_check=True)
                else:
                    def fill(n):
                        pass
                # ---- attention (window-major) ----
                aotg = {}
                for h in heads:
                    aotg[h] = aopool.tile([128, CHUNK], BF16, tag="aot",
                                          name=f"aot{h}")
                    aots.append(aotg[h])
                for w in range(NW):
                    for h in heads:
                        hi = h - g * HG
                        aot = aotg[h]
                        exps = []
                        for jb in range(8):
                            qlo = max(0, (jb - 4) * 128)
                            ncol = 512 - qlo
                            pd = psD.tile([128, 512], F32, tag="dt")
                            nc.tensor.matmul(
                                pd[:, :ncol],
                                kts[h][:, w * 512 + jb * 128: w * 512 + (jb + 1) * 128],
                                qts[h][:, w * 512 + qlo: w * 512 + 512],
                                start=True, stop=True)
                            ex = expool.tile([128, 512], BF16, tag="exp")
                            bias = w0b[:] if (w == 0 and jb < 4) else 0.0
                            nc.scalar.activation(ex[:, :ncol], pd[:, :ncol],
                                                 mybir.ActivationFunctionType.Exp,
                                                 bias=bias, scale=1.0)
                            if jb >= 4:
                                nc.vector.tensor_mul(ex[:, 0:128], ex[:, 0:128], tri[:])
                            exps.append((ex, qlo, ncol))
                        if g == NG - 1 and not (h == heads[0] and w == 0):
                            fill(4)
                        # row sums over kv: bf16 tree-add on DVE, then
                        # cross-partition add on gpsimd (frees the PE).
                        s01 = saccpool.tile([128, 512], BF16, tag="sacc")
                        nc.vector.tensor_add(s01[:], exps[0][0][:], exps[1][0][:])
                        s23 = saccpool.tile([128, 512], BF16, tag="sacc")
                        nc.vector.tensor_add(s23[:], exps[2][0][:], exps[3][0][:])
                        acc = saccpool.tile([128, 512], BF16, tag="sacc")
                        nc.vector.tensor_add(acc[:], s01[:], s23[:])
                        nc.vector.tensor_add(acc[:], acc[:], exps[4][0][:])
                        for jb in (5, 6, 7):
                            qlo = (jb - 4) * 128
                            nc.vector.tensor_add(acc[:, qlo:512], acc[:, qlo:512],
                                                 exps[jb][0][:, :512 - qlo])
                        sm = rbpool.tile([128, 512], F32, tag="sm")
                        nc.gpsimd.partition_all_reduce(sm[:], acc[:], 128,
                                                       bass_isa.ReduceOp.add)
                        rb = rbpool.tile([128, 512], F32, tag="rb")
                        nc.vector.reciprocal(rb[:], sm[:])
                        pav = psV.tile([128, 512], F32, tag="av")
                        for jb, (ex, qlo, ncol) in enumerate(exps):
                            t = w * 4 + jb
                            nc.tensor.matmul(pav[:, qlo:512],
                                             vns[t][:, hi * 128:(hi + 1) * 128],
                                             ex[:, :ncol],
                                             start=(jb == 0), stop=(jb == 7),
                                             skip_group_check=True)
                        nc.vector.tensor_mul(aot[:, w * 512:(w + 1) * 512], pav[:], rb[:])
                        if w == 0:
                            defer_vchain(hi)
            # ---- output projection (weights prefetched during last group) ----
            def out_tail(ft, blk, ps):
                ob = outpool.tile([128, 512], F32, tag="ob")
                nc.scalar.activation(ob[:], ps[:],
                                     mybir.ActivationFunctionType.Identity,
                                     bias=bo_sb[:, ft:ft + 1], scale=1.0)
                nc.sync.dma_start(
                    out_d[ft * 128:(ft + 1) * 128, blk * 512:(blk + 1) * 512], ob[:])

            # finish the split chains; the w0-half tail is ready ~4.5us
            # before the w1 half, so slot two w0-only chains in between to
            # keep the in-order PE stream busy across that window
            split = ET - 4
            for et in range(split, ET):
                nc.tensor.matmul(pss_split[0][:], wogs[0][:, et * 128:(et + 1) * 128],
                                 aots[et][:, 0:512],
                                 start=False, stop=(et == ET - 1),
                                 skip_group_check=True)
            out_tail(0, 0, pss_split[0])
            for ft in (1, 2, 3):
                ps = psP.tile([128, 512], F32, tag="proj", name=f"po{ft}b0")
                for et in range(ET):
                    nc.tensor.matmul(ps[:], wogs[ft][:, et * 128:(et + 1) * 128],
                                     aots[et][:, 0:512],
                                     start=(et == 0), stop=(et == ET - 1))
                out_tail(ft, 0, ps)
            for et in range(split, ET):
                nc.tensor.matmul(pss_split[1][:], wogs[0][:, et * 128:(et + 1) * 128],
                                 aots[et][:, 512:1024],
                                 start=False, stop=(et == ET - 1),
                                 skip_group_check=True)
            out_tail(0, 1, pss_split[1])
            for ft in range(1, ET):
                wog = wogs[ft]
                for blk in range(CHUNK // 512):
                    if blk == 0 and ft in (1, 2, 3):
                        continue
                    if ft == ET - 1 and blk == CHUNK // 512 - 1:
                        # final tile: two half-column chains so the first
                        # half's bias-add + store overlap the second half's
                        # matmuls instead of serializing after them
                        for col in range(2):
                            ps = psD.tile([128, 256], F32, tag="dt",
                                          name=f"pslast{col}")
                            cl = blk * 512 + col * 256
                            for et in range(ET):
                                nc.tensor.matmul(
                                    ps[:], wog[:, et * 128:(et + 1) * 128],
                                    aots[et][:, cl:cl + 256],
                                    start=(et == 0), stop=(et == ET - 1))
                            ob = outpool.tile([128, 256], F32, tag="ob",
                                              name=f"oblast{col}")
                            nc.scalar.activation(
                                ob[:], ps[:],
                                mybir.ActivationFunctionType.Identity,
                                bias=bo_sb[:, ft:ft + 1], scale=1.0)
                            nc.sync.dma_start(
                                out_d[ft * 128:(ft + 1) * 128, cl:cl + 256], ob[:])
                        continue
                    ps = psP.tile([128, 512], F32, tag="proj")
                    for et in range(ET):
                        nc.tensor.matmul(ps[:], wog[:, et * 128:(et + 1) * 128],
                                         aots[et][:, blk * 512:(blk + 1) * 512],
                                         start=(et == 0), stop=(et == ET - 1))
                    out_tail(ft, blk, ps)
    nc.compile()
    return nc


_NC_CACHE = None
_LAST_IN_MAPS = None


def _pack_whead(wt, scale=1.0):
    # wt = W.T as [E, E] f32; -> [128, H, E] with [p, h, et*128+d] = wt[et*128+p, h*128+d]
    r = (wt * scale).reshape(ET, 128, H, D).transpose(1, 2, 0, 3).reshape(128, H, E)
    return np.ascontiguousarray(r).astype(BF)


def kernel(query, key, value, input_mask, Wq, Wk, Wv, Wo, bo):
    global _NC_CACHE, _LAST_IN_MAPS
    if _NC_CACHE is None:
        _NC_CACHE = _build()
    nc = _NC_CACHE

    wq = _pack_whead(np.asarray(Wq, np.float32).T, SCALE)
    wk = _pack_whead(np.asarray(Wk, np.float32).T)
    wv = np.ascontiguousarray(np.asarray(Wv, np.float32).T).astype(BF)
    wo = _pack_whead(np.asarray(Wo, np.float32).T)          # [128, ft, et*128+f]
    bo_t = np.ascontiguousarray(
        np.asarray(bo, np.float32).reshape(ET, 128).T)      # [128, ET]
    tri = np.triu(np.ones((128, 128), np.float32)).astype(BF)

    in_maps = []
    for core in range(8):
        b, c = core // 4, core % 4
        lo, hi = c * CHUNK, (c + 1) * CHUNK
        xq = np.asarray(query[lo:hi, b, :], np.float32)     # [1024, E]
        xkv_k = np.zeros((TOK, E), np.float32)
        xkv_v = np.zeros((TOK, E), np.float32)
        klo = max(lo - HALO, 0)
        xkv_k[HALO - (lo - klo):] = np.asarray(key[klo:hi, b, :], np.float32)
        xkv_v[HALO - (lo - klo):] = np.asarray(value[klo:hi, b, :], np.float32)
        w0b = np.full((128, 1), NEG if c == 0 else 0.0, np.float32)
        in_maps.append({
            "xq": np.ascontiguousarray(xq.T).astype(BF),
            "xk": np.ascontiguousarray(xkv_k.T).astype(BF),
            "xv": np.ascontiguousarray(xkv_v.T).astype(BF),
            "wq": wq, "wk": wk, "wv": wv, "wo": wo,
            "bo": bo_t, "tri": tri, "w0b": w0b,
        })

    _LAST_IN_MAPS = in_maps
    res = bass_utils.run_bass_kernel_spmd(nc, in_maps, core_ids=list(range(8)))
    out = np.empty((S, B, E), np.float32)
    for core in range(8):
        b, c = core // 4, core % 4
        out[c * CHUNK:(c + 1) * CHUNK, b, :] = res.results[core]["out"].T
    return out
